# revision 5
# baseline (speedup 1.0000x reference)
"""2-layer GATv2 (PyG GATv2Conv semantics) on 8 Trainium2 NeuronCores.

v2 strategy (node-sharded, pull-gather, bf16):
  - Nodes sharded across 8 cores, balanced by degree; within a core, nodes are
    packed into NT=49 tiles of 128 balancing per-tile lo/hi in-edge counts so
    the global edge-tile count T = K_lo + K_hi is minimal.
  - Global xl table rows are laid out chunk-major ([chunk][core][tile][slot])
    so the AllGather can be split into 4 chunked collectives that overlap
    with projection compute.  The lo/hi half boundary (for int16 gather
    indices) falls on a chunk boundary.
  - Phase A: x^T comes pre-transposed from the host; per node tile, 6
    accumulating matmuls against [W1l|W1r] produce xl|xr in one PSUM tile.
    xr stays in SBUF; xl rows go to HBM and are chunk-AllGathered (bf16).
  - Phase B (layer-1 edges): per destination node tile, incoming edges are
    packed into T edge tiles of 128; source xl rows fetched with two
    dma_gathers (lo/hi halves).  One batched is_eq builds all T one-hot
    matrices; per edge tile the one-hot is PE-transposed to broadcast xr to
    edges, and the softmax numerator/denominator are accumulated with one
    one-hot matmul per edge tile.  All remaining elementwise work is batched
    over T on DVE/ACT in bf16 (fp32 PSUM/softmax denominators).
  - Layer-2 projections of (elu(h1)+1) (the +1 is corrected by a column-sum
    offset, turning elu into relu+min(exp,1) without a dual-scalar DVE op)
    are written to a 256-byte-padded table, chunk-AllGathered, and phase C
    repeats the edge pipeline with CO=16.
  - log_softmax per node on ACT/DVE, outputs written per tile.
"""

import sys

if "/opt/trn_rl_repo" not in sys.path:
    sys.path.insert(0, "/opt/trn_rl_repo")

import numpy as np
import ml_dtypes

NC = 8
P = 128
NEG_SLOPE = 0.2
NCHUNK = 2          # AllGather chunks == lo/hi halves (Shared: one writer each)
CHUNK_SPLIT = 1

_plan_cache = {}


# --------------------------------------------------------------------------
# host-side graph preprocessing
# --------------------------------------------------------------------------

def _snake(n, nbins):
    ids = np.arange(n)
    pos = ids % nbins
    return np.where((ids // nbins) % 2 == 0, pos, nbins - 1 - pos)


def _pack_idx(flat):
    n = len(flat)
    s = (n + 15) // 16
    arr = np.zeros(s * 16, np.int16)
    arr[:n] = flat
    return np.tile(arr.reshape(s, 16).T, (8, 1))


def _preprocess(N, E, edge_index):
    NPC = ((N + NC - 1) // NC + P - 1) // P * P
    NT = NPC // P
    TBL = NC * NPC

    # chunk tile counts (first chunks get the remainder)
    base, rem = divmod(NT, NCHUNK)
    ctiles = [base + (1 if i < rem else 0) for i in range(NCHUNK)]
    ct0 = np.cumsum([0] + ctiles)           # first tile of each chunk
    crow0 = [NC * P * int(ct0[i]) for i in range(NCHUNK)]   # global row base
    HALF = NC * P * int(ct0[CHUNK_SPLIT])
    assert HALF < 32768 and TBL - HALF < 32768
    lo_tiles = int(ct0[CHUNK_SPLIT])        # tiles 0..lo_tiles-1 are "lo"

    chunk_of_tile = np.zeros(NT, np.int64)
    for i in range(NCHUNK):
        chunk_of_tile[ct0[i]:ct0[i + 1]] = i

    crow0_a = np.array(crow0)
    ctiles_a = np.array(ctiles)
    ct0_a = np.array(ct0[:-1])

    def row_of(core, tl, slot):
        c = chunk_of_tile[tl]
        return (crow0_a[c] + core * (P * ctiles_a[c])
                + (tl - ct0_a[c]) * P + slot)

    src = np.concatenate([edge_index[0].astype(np.int64), np.arange(N)])
    dst = np.concatenate([edge_index[1].astype(np.int64), np.arange(N)])
    deg = np.bincount(dst, minlength=N)

    # core assignment: snake over degree-sorted nodes
    order = np.argsort(-deg, kind="stable")
    core_of = np.empty(N, np.int64)
    core_of[order] = _snake(N, NC)

    # pass 1: lo/hi group split per core (snake by degree, capacity-aware)
    lo_cap = lo_tiles * P
    hi_cap = (NT - lo_tiles) * P
    in_lo = np.zeros(N, bool)
    for c in range(NC):
        nodes = order[core_of[order] == c]          # degree-sorted
        nlo = min(lo_cap, max(len(nodes) - hi_cap,
                              (len(nodes) * lo_cap + lo_cap + hi_cap - 1)
                              // (lo_cap + hi_cap)))
        sel = _snake(len(nodes), 2) == 0
        # fix capacity: take first nlo by snake, overflow to other group
        lo_n = nodes[sel][:lo_cap]
        hi_n = nodes[~sel][:hi_cap]
        rest = np.concatenate([nodes[sel][lo_cap:], nodes[~sel][hi_cap:]])
        free_lo = lo_cap - len(lo_n)
        lo_n = np.concatenate([lo_n, rest[:free_lo]])
        hi_n = np.concatenate([hi_n, rest[free_lo:]])
        in_lo[lo_n.astype(np.int64)] = True

    lo_src = in_lo[src]
    deg_lo = np.bincount(dst[lo_src], minlength=N)
    deg_hi = deg - deg_lo

    # pass 2: per core per group, greedy pack nodes into tiles balancing
    # (lo, hi) in-edge loads, capacity 128 nodes per tile
    local_of = np.empty(N, np.int64)
    tiles_nodes = {}
    for c in range(NC):
        for grp, (t0, t1) in (("lo", (0, lo_tiles)), ("hi", (lo_tiles, NT))):
            nodes = np.where((core_of == c) & (in_lo == (grp == "lo")))[0]
            w = deg_lo[nodes] + deg_hi[nodes]
            nodes = nodes[np.argsort(-w, kind="stable")]
            ntile = t1 - t0
            llo = np.zeros(ntile)
            lhi = np.zeros(ntile)
            cnt = np.zeros(ntile, np.int64)
            tl_of = np.empty(len(nodes), np.int64)
            for i, v in enumerate(nodes):
                load = np.maximum(llo + deg_lo[v], lhi + deg_hi[v]) \
                    + 0.001 * (llo + lhi)
                load[cnt >= P] = np.inf
                t = int(np.argmin(load))
                tl_of[i] = t
                llo[t] += deg_lo[v]
                lhi[t] += deg_hi[v]
                cnt[t] += 1
            slot = np.zeros(ntile, np.int64)
            for i, v in enumerate(nodes):
                t = tl_of[i]
                local_of[v] = (t0 + t) * P + slot[t]
                slot[t] += 1
            for t in range(ntile):
                tiles_nodes[(c, t0 + t)] = nodes[tl_of == t]

    r_of = row_of(core_of, local_of // P, local_of % P)

    # per (core, tile) edge lists split by half
    e_core = core_of[dst]
    e_tile = local_of[dst] // P
    e_slot = local_of[dst] % P
    lists = {}
    K_lo = K_hi = 1
    for c in range(NC):
        m_c = e_core == c
        for tl in range(NT):
            m = m_c & (e_tile == tl)
            ml = m & lo_src
            mh = m & ~lo_src
            rl, sl_ = r_of[src[ml]], e_slot[ml]
            rh, sh_ = r_of[src[mh]] - HALF, e_slot[mh]
            npad = P - len(tiles_nodes[(c, tl)])
            if npad:
                pads = np.arange(P - npad, P)
                rl = np.concatenate([rl, np.zeros(npad, np.int64)])
                sl_ = np.concatenate([sl_, pads])
            lists[(c, tl)] = (rl, sl_, rh, sh_)
            K_lo = max(K_lo, (len(rl) + P - 1) // P)
            K_hi = max(K_hi, (len(rh) + P - 1) // P)
    T = K_lo + K_hi
    assert K_lo * P <= 1024 and K_hi * P <= 1024, (K_lo, K_hi)

    gidx = np.zeros((NC, P, NT * T * 8), np.int16)
    drel = np.full((NC, P, NT * T), -1.0, np.float32)
    for c in range(NC):
        for tl in range(NT):
            rl, sl_, rh, sh_ = lists[(c, tl)]
            for half, K, rows, slots in ((0, K_lo, rl, sl_),
                                         (1, K_hi, rh, sh_)):
                n = len(rows)
                flat = np.zeros(K * P, np.int64)
                flat[:n] = rows
                off = tl * T * 8 + (K_lo * 8 if half else 0)
                gidx[c, :, off:off + K * 8] = _pack_idx(flat)
                dr = np.full(K * P, -1.0, np.float32)
                dr[:n] = slots
                tc0 = tl * T + (K_lo if half else 0)
                drel[c, :, tc0:tc0 + K] = dr.reshape(K, P).T

    node_order = np.full((NC, NPC), -1, np.int64)
    for c in range(NC):
        nodes = np.where(core_of == c)[0]
        node_order[c, local_of[nodes]] = nodes

    return dict(NPC=NPC, NT=NT, TBL=TBL, HALF=HALF, K_lo=K_lo, K_hi=K_hi,
                T=T, ctiles=ctiles, ct0=[int(x) for x in ct0], crow0=crow0,
                gidx=gidx, drel=drel, node_order=node_order,
                core_of=core_of, local_of=local_of)


# --------------------------------------------------------------------------
# bass program
# --------------------------------------------------------------------------

def _build_program(dims):
    import concourse.bass as bass
    import concourse.mybir as mybir
    import concourse.tile as tile
    from concourse import library_config
    from concourse.bass import _add_dep_helper
    import bass_rust as _br

    fp32 = mybir.dt.float32
    bf = mybir.dt.bfloat16
    i16 = mybir.dt.int16
    AX = mybir.AxisListType
    OP = mybir.AluOpType
    AF = mybir.ActivationFunctionType

    DIN = dims["DIN"]; HC = dims["HC"]; H = dims["H"]; CH = dims["CH"]
    CO = dims["CO"]
    NPC = dims["NPC"]; NT = dims["NT"]; TBL = dims["TBL"]
    HALF = dims["HALF"]
    K_lo = dims["K_lo"]; K_hi = dims["K_hi"]; T = dims["T"]
    ctiles = dims["ctiles"]; ct0 = dims["ct0"]; crow0 = dims["crow0"]
    KD = DIN // P
    KH = HC // P
    COP = 128                       # layer-2 table row = 256B (bf16)

    nc = bass.Bass(num_devices=NC)

    xT = nc.dram_tensor("xT", [P, KD, NPC], bf, kind="ExternalInput")
    w1 = nc.dram_tensor("w1", [P, KD, 2 * HC], bf, kind="ExternalInput")
    w2 = nc.dram_tensor("w2", [P, KH, 2 * CO], bf, kind="ExternalInput")
    CCOLS = P + P + HC + CO + HC + CO + 2 * CO
    consts = nc.dram_tensor("consts", [P, CCOLS], bf, kind="ExternalInput")
    constf = nc.dram_tensor("constf", [P, 1], fp32, kind="ExternalInput")
    gidx_d = nc.dram_tensor("gidx", [P, NT * T * 8], i16, kind="ExternalInput")
    drel_d = nc.dram_tensor("drel", [P, NT * T], bf, kind="ExternalInput")
    h2_out = nc.dram_tensor("h2o", [NPC, CO], fp32, kind="ExternalOutput")
    ls_out = nc.dram_tensor("lso", [NPC, CO], fp32, kind="ExternalOutput")

    with tile.TileContext(nc) as tc:
        with (
            tc.tile_pool(name="dram", bufs=1, space="DRAM") as dram,
            tc.tile_pool(name="cst", bufs=1) as cst,
        ):
            lib = nc.gpsimd.load_library(library_config.mlp)
            reg_klo = nc.gpsimd.to_reg(K_lo * P)
            reg_khi = nc.gpsimd.to_reg(K_hi * P)

            ctile = cst.tile([P, CCOLS], bf)
            nc.sync.dma_start(out=ctile[:], in_=consts[:])
            cftile = cst.tile([P, 1], fp32)
            nc.sync.dma_start(out=cftile[:], in_=constf[:])
            o = 0
            iota = ctile[:, o:o + P]; o += P
            ident = ctile[:, o:o + P]; o += P
            attB = ctile[:, o:o + HC]; o += HC
            att2B = ctile[:, o:o + CO]; o += CO
            b1B = ctile[:, o:o + HC]; o += HC
            b2B = ctile[:, o:o + CO]; o += CO
            w2sB = ctile[:, o:o + 2 * CO]; o += 2 * CO
            alpha = cftile[:, 0:1]

            w1_sb = cst.tile([P, KD, 2 * HC], bf)
            nc.sync.dma_start(out=w1_sb[:], in_=w1[:])
            w2_sb = cst.tile([P, KH, 2 * CO], bf)
            nc.sync.dma_start(out=w2_sb[:], in_=w2[:])
            gidx_sb = cst.tile([P, NT * T * 8], i16)
            nc.sync.dma_start(out=gidx_sb[:], in_=gidx_d[:])
            drel_sb = cst.tile([P, NT * T], bf)
            nc.sync.dma_start(out=drel_sb[:], in_=drel_d[:])

            xr1_all = cst.tile([P, NT, HC], bf)
            xr2_all = cst.tile([P, NT, CO], bf)

            ag1c = [dram.tile([ctiles[i] * P, HC], bf, name=f"ag1c{i}")
                    for i in range(NCHUNK)]
            ag2c = [dram.tile([ctiles[i] * P, COP], bf, name=f"ag2c{i}")
                    for i in range(NCHUNK)]
            tbl1h = [dram.tile([NC * ctiles[i] * P, HC], bf,
                                addr_space="Shared", name=f"tbl1h{i}")
                     for i in range(NCHUNK)]
            tbl2h = [dram.tile([NC * ctiles[i] * P, COP], bf,
                                addr_space="Shared", name=f"tbl2h{i}")
                     for i in range(NCHUNK)]

            def ag_chunk(i, agc, tblh):
                nc.gpsimd.collective_compute(
                    "AllGather", mybir.AluOpType.bypass,
                    replica_groups=[list(range(NC))],
                    ins=[agc[i][:].opt()], outs=[tblh[i][:].opt()],
                )

            # ================= phase A: layer-1 projections =================
            with (tc.tile_pool(name="sbA", bufs=3) as sb,
                  tc.tile_pool(name="psA", bufs=2, space="PSUM") as ps):
                for i in range(NCHUNK):
                    for tl in range(ct0[i], ct0[i + 1]):
                        xt = sb.tile([P, KD, P], bf, tag="xt")
                        nc.sync.dma_start(
                            out=xt[:], in_=xT[:, :, tl * P:(tl + 1) * P])
                        pA = ps.tile([P, 2 * HC], fp32, tag="mm", space="PSUM")
                        for k in range(KD):
                            nc.tensor.matmul(out=pA[:], lhsT=xt[:, k, :],
                                             rhs=w1_sb[:, k, :],
                                             start=(k == 0), stop=(k == KD - 1))
                        xlr = sb.tile([P, 2 * HC], bf, tag="xlr")
                        nc.vector.tensor_copy(out=xlr[:], in_=pA[:])
                        nc.vector.tensor_copy(out=xr1_all[:, tl, :],
                                              in_=xlr[:, HC:2 * HC])
                        lt = tl - ct0[i]
                        nc.sync.dma_start(
                            out=ag1c[i][lt * P:(lt + 1) * P, :],
                            in_=xlr[:, 0:HC])
                    ag_chunk(i, ag1c, tbl1h)

            # ================= phase B: layer-1 edges =======================
            with (tc.tile_pool(name="sbB", bufs=2) as sb,
                  tc.tile_pool(name="psB", bufs=2, space="PSUM") as ps):
                for i in range(NCHUNK):
                    for tl in range(ct0[i], ct0[i + 1]):
                        nt = tl
                        g = sb.tile([P, T, HC], bf, tag="g")
                        off = nt * T * 8
                        g1 = nc.gpsimd.dma_gather(
                            g[:, 0:K_lo, :], tbl1h[0][:],
                            gidx_sb[:, off:off + K_lo * 8],
                            K_lo * P, reg_klo, HC)
                        g2 = nc.gpsimd.dma_gather(
                            g[:, K_lo:T, :], tbl1h[1][:],
                            gidx_sb[:, off + K_lo * 8:off + T * 8],
                            K_hi * P, reg_khi, HC)
                        _add_dep_helper(g1.ins, lib.ins, sync=False, reason="lib")
                        _add_dep_helper(g2.ins, lib.ins, sync=False, reason="lib")

                        oh = sb.tile([P, T, P], bf, tag="oh")
                        nc.vector.tensor_tensor(
                            out=oh[:],
                            in0=iota[:, None, :].to_broadcast([P, T, P]),
                            in1=drel_sb[:, nt * T:(nt + 1) * T, None]
                                .to_broadcast([P, T, P]),
                            op=OP.is_equal)

                        z = sb.tile([P, T, HC], bf, tag="z")
                        for t in range(T):
                            tp_ps = ps.tile([P, P], bf, tag="tp", space="PSUM")
                            nc.tensor.transpose(tp_ps[:], oh[:, t, :], ident)
                            ohn = sb.tile([P, P], bf, tag="ohn")
                            nc.scalar.copy(out=ohn[:], in_=tp_ps[:])
                            zr_ps = ps.tile([P, HC], fp32, tag="zr", space="PSUM")
                            nc.tensor.matmul(out=zr_ps[:], lhsT=ohn[:],
                                             rhs=xr1_all[:, nt, :],
                                             start=True, stop=True)
                            nc.vector.tensor_tensor(
                                out=z[:, t, :], in0=g[:, t, :], in1=zr_ps[:],
                                op=OP.add)

                        pr = sb.tile([P, T, HC], bf, tag="pr")
                        nc.scalar.activation(out=pr[:], in_=z[:], func=AF.Prelu,
                                             alpha=alpha)
                        ta = sb.tile([P, T, HC], bf, tag="ta")
                        nc.vector.tensor_tensor(
                            out=ta[:], in0=pr[:],
                            in1=attB[:, None, :].to_broadcast([P, T, HC]),
                            op=OP.mult)
                        sc = sb.tile([P, T, H], fp32, tag="sc")
                        nc.vector.tensor_reduce(
                            out=sc[:],
                            in_=ta[:].rearrange("p t (h c) -> p t h c", h=H),
                            axis=AX.X, op=OP.add)
                        msg = sb.tile([P, T, HC + H], bf, tag="msg")
                        nc.scalar.activation(out=msg[:, :, HC:HC + H], in_=sc[:],
                                             func=AF.Exp)
                        nc.vector.tensor_tensor(
                            out=msg[:, :, 0:HC]
                                .rearrange("p t (h c) -> p t h c", h=H),
                            in0=g[:].rearrange("p t (h c) -> p t h c", h=H),
                            in1=msg[:, :, HC:HC + H][:, :, :, None]
                                .to_broadcast([P, T, H, CH]),
                            op=OP.mult)

                        acc = ps.tile([P, HC + H], fp32, tag="acc", space="PSUM")
                        for t in range(T):
                            nc.tensor.matmul(out=acc[:], lhsT=oh[:, t, :],
                                             rhs=msg[:, t, :],
                                             start=(t == 0), stop=(t == T - 1))

                        rec = sb.tile([P, H], fp32, tag="rec")
                        nc.vector.reciprocal(out=rec[:], in_=acc[:, HC:HC + H])
                        h1 = sb.tile([P, HC], bf, tag="h1")
                        nc.vector.tensor_tensor(
                            out=h1[:].rearrange("p (h c) -> p h c", h=H),
                            in0=acc[:, 0:HC].rearrange("p (h c) -> p h c", h=H),
                            in1=rec[:, :, None].to_broadcast([P, H, CH]),
                            op=OP.mult)
                        if dims["add_b1"]:
                            nc.vector.tensor_tensor(out=h1[:], in0=h1[:],
                                                    in1=b1B, op=OP.add)
                        # elu(h1)+1 = relu(h1) + min(exp(h1), 1)
                        el = sb.tile([P, HC], bf, tag="el")
                        nc.scalar.activation(out=el[:], in_=h1[:], func=AF.Relu)
                        ev = sb.tile([P, HC], bf, tag="ev")
                        nc.scalar.activation(out=ev[:], in_=h1[:], func=AF.Exp)
                        nc.vector.tensor_scalar(out=ev[:], in0=ev[:],
                                                scalar1=1.0, scalar2=None,
                                                op0=OP.min)
                        nc.vector.tensor_tensor(out=el[:], in0=el[:], in1=ev[:],
                                                op=OP.add)

                        hT = sb.tile([P, KH, P], bf, tag="hT")
                        for k in range(KH):
                            tp_ps = ps.tile([P, P], bf, tag="tp", space="PSUM")
                            nc.tensor.transpose(tp_ps[:],
                                                el[:, k * P:(k + 1) * P], ident)
                            nc.vector.tensor_copy(out=hT[:, k, :], in_=tp_ps[:])
                        p2 = ps.tile([P, 2 * CO], fp32, tag="p2", space="PSUM")
                        for k in range(KH):
                            nc.tensor.matmul(out=p2[:], lhsT=hT[:, k, :],
                                             rhs=w2_sb[:, k, :],
                                             start=(k == 0), stop=(k == KH - 1))
                        xlr2 = sb.tile([P, 2 * CO], bf, tag="xlr2")
                        nc.vector.tensor_tensor(out=xlr2[:], in0=p2[:],
                                                in1=w2sB, op=OP.subtract)
                        nc.vector.tensor_copy(out=xr2_all[:, nt, :],
                                              in_=xlr2[:, CO:2 * CO])
                        lt = tl - ct0[i]
                        nc.sync.dma_start(
                            out=ag2c[i][lt * P:(lt + 1) * P, 0:CO],
                            in_=xlr2[:, 0:CO])
                    ag_chunk(i, ag2c, tbl2h)

            # ================= phase C: layer-2 edges =======================
            with (tc.tile_pool(name="sbC", bufs=2) as sb,
                  tc.tile_pool(name="psC", bufs=2, space="PSUM") as ps):
                for nt in range(NT):
                    g2t = sb.tile([P, T, COP], bf, tag="g2")
                    off = nt * T * 8
                    g1 = nc.gpsimd.dma_gather(
                        g2t[:, 0:K_lo, :], tbl2h[0][:],
                        gidx_sb[:, off:off + K_lo * 8],
                        K_lo * P, reg_klo, COP)
                    g2 = nc.gpsimd.dma_gather(
                        g2t[:, K_lo:T, :], tbl2h[1][:],
                        gidx_sb[:, off + K_lo * 8:off + T * 8],
                        K_hi * P, reg_khi, COP)
                    _add_dep_helper(g1.ins, lib.ins, sync=False, reason="lib")
                    _add_dep_helper(g2.ins, lib.ins, sync=False, reason="lib")

                    oh = sb.tile([P, T, P], bf, tag="oh")
                    nc.vector.tensor_tensor(
                        out=oh[:],
                        in0=iota[:, None, :].to_broadcast([P, T, P]),
                        in1=drel_sb[:, nt * T:(nt + 1) * T, None]
                            .to_broadcast([P, T, P]),
                        op=OP.is_equal)

                    z2 = sb.tile([P, T, CO], bf, tag="z2")
                    for t in range(T):
                        tp_ps = ps.tile([P, P], bf, tag="tp", space="PSUM")
                        nc.tensor.transpose(tp_ps[:], oh[:, t, :], ident)
                        ohn = sb.tile([P, P], bf, tag="ohn")
                        nc.scalar.copy(out=ohn[:], in_=tp_ps[:])
                        zr_ps = ps.tile([P, CO], fp32, tag="zr", space="PSUM")
                        nc.tensor.matmul(out=zr_ps[:], lhsT=ohn[:],
                                         rhs=xr2_all[:, nt, :],
                                         start=True, stop=True)
                        nc.vector.tensor_tensor(
                            out=z2[:, t, :], in0=g2t[:, t, 0:CO], in1=zr_ps[:],
                            op=OP.add)

                    pr2 = sb.tile([P, T, CO], bf, tag="pr2")
                    nc.scalar.activation(out=pr2[:], in_=z2[:], func=AF.Prelu,
                                         alpha=alpha)
                    ta2 = sb.tile([P, T, CO], bf, tag="ta2")
                    nc.vector.tensor_tensor(
                        out=ta2[:], in0=pr2[:],
                        in1=att2B[:, None, :].to_broadcast([P, T, CO]),
                        op=OP.mult)
                    sc2 = sb.tile([P, T], fp32, tag="sc2")
                    nc.vector.tensor_reduce(out=sc2[:], in_=ta2[:],
                                            axis=AX.X, op=OP.add)
                    msg2 = sb.tile([P, T, CO + 1], bf, tag="msg2")
                    nc.scalar.activation(out=msg2[:, :, CO:CO + 1],
                                         in_=sc2[:, :, None], func=AF.Exp)
                    nc.vector.tensor_tensor(
                        out=msg2[:, :, 0:CO], in0=g2t[:, :, 0:CO],
                        in1=msg2[:, :, CO:CO + 1].to_broadcast([P, T, CO]),
                        op=OP.mult)

                    acc2 = ps.tile([P, CO + 1], fp32, tag="acc", space="PSUM")
                    for t in range(T):
                        nc.tensor.matmul(out=acc2[:], lhsT=oh[:, t, :],
                                         rhs=msg2[:, t, :],
                                         start=(t == 0), stop=(t == T - 1))

                    rec2 = sb.tile([P, 1], fp32, tag="rec2")
                    nc.vector.reciprocal(out=rec2[:], in_=acc2[:, CO:CO + 1])
                    h2 = sb.tile([P, CO], fp32, tag="h2")
                    nc.vector.tensor_scalar(out=h2[:], in0=acc2[:, 0:CO],
                                            scalar1=rec2[:, 0:1], scalar2=None,
                                            op0=OP.mult)
                    if dims["add_b2"]:
                        nc.vector.tensor_tensor(out=h2[:], in0=h2[:], in1=b2B,
                                                op=OP.add)
                    nc.sync.dma_start(out=h2_out[nt * P:(nt + 1) * P, :],
                                      in_=h2[:])
                    nm = sb.tile([P, 1], fp32, tag="nm")
                    nc.vector.tensor_reduce(out=nm[:], in_=h2[:], axis=AX.X,
                                            op=OP.max, negate=True)
                    esc = sb.tile([P, CO], fp32, tag="esc")
                    ssum = sb.tile([P, 1], fp32, tag="ssum")
                    nc.scalar.activation(out=esc[:], in_=h2[:], func=AF.Exp,
                                         bias=nm[:, 0:1],
                                         accum_out=ssum[:, 0:1])
                    lns = sb.tile([P, 1], fp32, tag="lns")
                    nc.scalar.activation(out=lns[:], in_=ssum[:], func=AF.Ln)
                    ls = sb.tile([P, CO], fp32, tag="ls")
                    nc.vector.tensor_scalar(out=ls[:], in0=h2[:],
                                            scalar1=nm[:, 0:1], scalar2=None,
                                            op0=OP.add)
                    nc.vector.tensor_scalar(out=ls[:], in0=ls[:],
                                            scalar1=lns[:, 0:1], scalar2=None,
                                            op0=OP.subtract)
                    nc.sync.dma_start(out=ls_out[nt * P:(nt + 1) * P, :],
                                      in_=ls[:])

    _br.generate_event_semaphores(nc)
    _br.codegen_inst_isa_subclasses(nc)
    return nc


# --------------------------------------------------------------------------
# entry point
# --------------------------------------------------------------------------

def kernel(x, edge_index, W1l, W1r, att1, b1, W2l, W2r, att2, b2):
    x = np.asarray(x, np.float32)
    edge_index = np.asarray(edge_index)
    W1l = np.asarray(W1l, np.float32); W1r = np.asarray(W1r, np.float32)
    att1 = np.asarray(att1, np.float32); b1 = np.asarray(b1, np.float32)
    W2l = np.asarray(W2l, np.float32); W2r = np.asarray(W2r, np.float32)
    att2 = np.asarray(att2, np.float32); b2 = np.asarray(b2, np.float32)

    N, DIN = x.shape
    E = edge_index.shape[1]
    H, CH = att1.shape
    HC = W1l.shape[1]
    CO = W2l.shape[1]
    KD = DIN // P
    KH = HC // P

    key = (N, E, DIN, H, CH, HC, CO,
           int(np.abs(b1).max() > 0), int(np.abs(b2).max() > 0),
           hash(edge_index.tobytes()))
    if key in _plan_cache:
        pp, nc, dims = _plan_cache[key]
    else:
        pp = _preprocess(N, E, edge_index)
        dims = dict(DIN=DIN, HC=HC, H=H, CH=CH, CO=CO,
                    NPC=pp["NPC"], NT=pp["NT"], TBL=pp["TBL"],
                    HALF=pp["HALF"], K_lo=pp["K_lo"], K_hi=pp["K_hi"],
                    T=pp["T"], ctiles=pp["ctiles"], ct0=pp["ct0"],
                    crow0=pp["crow0"],
                    add_b1=bool(np.abs(b1).max() > 0),
                    add_b2=bool(np.abs(b2).max() > 0))
        nc = _build_program(dims)
        _plan_cache[key] = (pp, nc, dims)

    NPC = pp["NPC"]
    bfdt = ml_dtypes.bfloat16

    iota = np.broadcast_to(np.arange(P, dtype=np.float32)[None, :], (P, P))
    ident = np.eye(P, dtype=np.float32)
    attB = np.broadcast_to(att1.reshape(1, HC), (P, HC))
    att2B = np.broadcast_to(att2.reshape(1, CO), (P, CO))
    b1B = np.broadcast_to(b1.reshape(1, HC), (P, HC))
    b2B = np.broadcast_to(b2.reshape(1, CO), (P, CO))
    w2s = np.concatenate([W2l.sum(axis=0), W2r.sum(axis=0)])
    w2sB = np.broadcast_to(w2s.reshape(1, 2 * CO), (P, 2 * CO))
    consts = np.concatenate([iota, ident, attB, att2B, b1B, b2B, w2sB],
                            axis=1).astype(bfdt)
    constf = np.full((P, 1), NEG_SLOPE, np.float32)

    w1p = np.concatenate([W1l.reshape(KD, P, HC).transpose(1, 0, 2),
                          W1r.reshape(KD, P, HC).transpose(1, 0, 2)],
                         axis=2).astype(bfdt)       # [P, KD, 2HC]
    w2p = np.concatenate([W2l.reshape(KH, P, CO).transpose(1, 0, 2),
                          W2r.reshape(KH, P, CO).transpose(1, 0, 2)],
                         axis=2).astype(bfdt)       # [P, KH, 2CO]

    in_maps = []
    for c in range(NC):
        sel = pp["node_order"][c]
        real = sel >= 0
        xa = np.zeros((NPC, DIN), np.float32)
        xa[real] = x[sel[real]]
        xTc = np.ascontiguousarray(
            xa.T.reshape(KD, P, NPC).transpose(1, 0, 2)).astype(bfdt)
        in_maps.append(dict(
            xT=xTc, w1=w1p, w2=w2p, consts=consts, constf=constf,
            gidx=np.ascontiguousarray(pp["gidx"][c]),
            drel=np.ascontiguousarray(pp["drel"][c]).astype(bfdt),
        ))

    from concourse.bass_utils import run_bass_kernel_spmd
    res = run_bass_kernel_spmd(nc, in_maps, core_ids=list(range(NC)))

    h = np.empty((N, CO), np.float32)
    ls = np.empty((N, CO), np.float32)
    r_core = pp["core_of"]
    r_loc = pp["local_of"]
    for c in range(NC):
        m = r_core == c
        h[m] = res.results[c]["h2o"][r_loc[m]]
        ls[m] = res.results[c]["lso"][r_loc[m]]
    return h, ls


# revision 9
# speedup vs baseline: 1.1888x; 1.1888x over previous
"""2-layer GATv2 (PyG GATv2Conv semantics) on 8 Trainium2 NeuronCores.

v2 strategy (node-sharded, pull-gather, bf16):
  - Nodes sharded across 8 cores, balanced by degree; within a core, nodes are
    packed into NT=49 tiles of 128 balancing per-tile lo/hi in-edge counts so
    the global edge-tile count T = K_lo + K_hi is minimal.
  - Global xl table rows are laid out chunk-major ([chunk][core][tile][slot])
    so the AllGather can be split into 4 chunked collectives that overlap
    with projection compute.  The lo/hi half boundary (for int16 gather
    indices) falls on a chunk boundary.
  - Phase A: x^T comes pre-transposed from the host; per node tile, 6
    accumulating matmuls against [W1l|W1r] produce xl|xr in one PSUM tile.
    xr stays in SBUF; xl rows go to HBM and are chunk-AllGathered (bf16).
  - Phase B (layer-1 edges): per destination node tile, incoming edges are
    packed into T edge tiles of 128; source xl rows fetched with two
    dma_gathers (lo/hi halves).  One batched is_eq builds all T one-hot
    matrices; per edge tile the one-hot is PE-transposed to broadcast xr to
    edges, and the softmax numerator/denominator are accumulated with one
    one-hot matmul per edge tile.  All remaining elementwise work is batched
    over T on DVE/ACT in bf16 (fp32 PSUM/softmax denominators).
  - Layer-2 projections of (elu(h1)+1) (the +1 is corrected by a column-sum
    offset, turning elu into relu+min(exp,1) without a dual-scalar DVE op)
    are written to a 256-byte-padded table, chunk-AllGathered, and phase C
    repeats the edge pipeline with CO=16.
  - log_softmax per node on ACT/DVE, outputs written per tile.
"""

import sys

if "/opt/trn_rl_repo" not in sys.path:
    sys.path.insert(0, "/opt/trn_rl_repo")

import numpy as np
import ml_dtypes

NC = 8
P = 128
NEG_SLOPE = 0.2
NCHUNK = 2          # AllGather chunks == lo/hi halves (Shared: one writer each)
CHUNK_SPLIT = 1

_plan_cache = {}


# --------------------------------------------------------------------------
# host-side graph preprocessing
# --------------------------------------------------------------------------

def _snake(n, nbins):
    ids = np.arange(n)
    pos = ids % nbins
    return np.where((ids // nbins) % 2 == 0, pos, nbins - 1 - pos)


def _pack_idx(flat):
    n = len(flat)
    s = (n + 15) // 16
    arr = np.zeros(s * 16, np.int16)
    arr[:n] = flat
    return np.tile(arr.reshape(s, 16).T, (8, 1))


def _preprocess(N, E, edge_index):
    NPC = ((N + NC - 1) // NC + P - 1) // P * P
    NT = NPC // P
    TBL = NC * NPC

    # chunk tile counts (first chunks get the remainder)
    base, rem = divmod(NT, NCHUNK)
    ctiles = [base + (1 if i < rem else 0) for i in range(NCHUNK)]
    ct0 = np.cumsum([0] + ctiles)           # first tile of each chunk
    crow0 = [NC * P * int(ct0[i]) for i in range(NCHUNK)]   # global row base
    HALF = NC * P * int(ct0[CHUNK_SPLIT])
    assert HALF < 32768 and TBL - HALF < 32768
    lo_tiles = int(ct0[CHUNK_SPLIT])        # tiles 0..lo_tiles-1 are "lo"

    chunk_of_tile = np.zeros(NT, np.int64)
    for i in range(NCHUNK):
        chunk_of_tile[ct0[i]:ct0[i + 1]] = i

    crow0_a = np.array(crow0)
    ctiles_a = np.array(ctiles)
    ct0_a = np.array(ct0[:-1])

    def row_of(core, tl, slot):
        c = chunk_of_tile[tl]
        return (crow0_a[c] + core * (P * ctiles_a[c])
                + (tl - ct0_a[c]) * P + slot)

    # self-loops are handled by a separate local (affine) path on-device;
    # the edge lists hold only the real edges
    src = edge_index[0].astype(np.int64)
    dst = edge_index[1].astype(np.int64)
    deg = np.bincount(dst, minlength=N)

    # core assignment: snake over degree-sorted nodes
    order = np.argsort(-deg, kind="stable")
    core_of = np.empty(N, np.int64)
    core_of[order] = _snake(N, NC)

    # pass 1: lo/hi group split per core (snake by degree, capacity-aware)
    lo_cap = lo_tiles * P
    hi_cap = (NT - lo_tiles) * P
    in_lo = np.zeros(N, bool)
    for c in range(NC):
        nodes = order[core_of[order] == c]          # degree-sorted
        nlo = min(lo_cap, max(len(nodes) - hi_cap,
                              (len(nodes) * lo_cap + lo_cap + hi_cap - 1)
                              // (lo_cap + hi_cap)))
        sel = _snake(len(nodes), 2) == 0
        # fix capacity: take first nlo by snake, overflow to other group
        lo_n = nodes[sel][:lo_cap]
        hi_n = nodes[~sel][:hi_cap]
        rest = np.concatenate([nodes[sel][lo_cap:], nodes[~sel][hi_cap:]])
        free_lo = lo_cap - len(lo_n)
        lo_n = np.concatenate([lo_n, rest[:free_lo]])
        hi_n = np.concatenate([hi_n, rest[free_lo:]])
        in_lo[lo_n.astype(np.int64)] = True

    lo_src = in_lo[src]
    deg_lo = np.bincount(dst[lo_src], minlength=N)
    deg_hi = deg - deg_lo

    # pass 2: per core per group, greedy pack nodes into tiles balancing
    # (lo, hi) in-edge loads, capacity 128 nodes per tile
    local_of = np.empty(N, np.int64)
    tiles_nodes = {}
    for c in range(NC):
        for grp, (t0, t1) in (("lo", (0, lo_tiles)), ("hi", (lo_tiles, NT))):
            nodes = np.where((core_of == c) & (in_lo == (grp == "lo")))[0]
            w = deg_lo[nodes] + deg_hi[nodes]
            nodes = nodes[np.argsort(-w, kind="stable")]
            ntile = t1 - t0
            llo = np.zeros(ntile)
            lhi = np.zeros(ntile)
            cnt = np.zeros(ntile, np.int64)
            tl_of = np.empty(len(nodes), np.int64)
            for i, v in enumerate(nodes):
                load = np.maximum(llo + deg_lo[v], lhi + deg_hi[v]) \
                    + 0.001 * (llo + lhi)
                load[cnt >= P] = np.inf
                t = int(np.argmin(load))
                tl_of[i] = t
                llo[t] += deg_lo[v]
                lhi[t] += deg_hi[v]
                cnt[t] += 1
            slot = np.zeros(ntile, np.int64)
            for i, v in enumerate(nodes):
                t = tl_of[i]
                local_of[v] = (t0 + t) * P + slot[t]
                slot[t] += 1
            for t in range(ntile):
                tiles_nodes[(c, t0 + t)] = nodes[tl_of == t]

    r_of = row_of(core_of, local_of // P, local_of % P)

    # per (core, tile) edge lists split by half
    e_core = core_of[dst]
    e_tile = local_of[dst] // P
    e_slot = local_of[dst] % P
    lists = {}
    K_lo = K_hi = 1
    for c in range(NC):
        m_c = e_core == c
        for tl in range(NT):
            m = m_c & (e_tile == tl)
            ml = m & lo_src
            mh = m & ~lo_src
            rl, sl_ = r_of[src[ml]], e_slot[ml]
            rh, sh_ = r_of[src[mh]] - HALF, e_slot[mh]
            lists[(c, tl)] = (rl, sl_, rh, sh_)
            K_lo = max(K_lo, (len(rl) + P - 1) // P)
            K_hi = max(K_hi, (len(rh) + P - 1) // P)
    # swap-repair: try to shrink K_lo/K_hi by one via node swaps between
    # tiles of the same (core, group)  (group swaps would change edge labels)
    def tile_counts():
        clo = np.zeros((NC, NT), np.int64)
        chi = np.zeros((NC, NT), np.int64)
        for c in range(NC):
            for tl in range(NT):
                rl, _, rh, _ = lists[(c, tl)]
                clo[c, tl] = len(rl)
                chi[c, tl] = len(rh)
        return clo, chi

    def try_repair(cap_lo, cap_hi):
        moved = False
        for c in range(NC):
            for t0, t1 in ((0, lo_tiles), (lo_tiles, NT)):
                tset = list(range(t0, t1))
                nt_nodes = {t: list(tiles_nodes[(c, t)]) for t in tset}
                Llo = {t: deg_lo[nt_nodes[t]].sum() for t in tset}
                Lhi = {t: deg_hi[nt_nodes[t]].sum() for t in tset}
                for _ in range(4000):
                    viols = [(max(Llo[t] - cap_lo, Lhi[t] - cap_hi), t)
                             for t in tset]
                    v, tA = max(viols)
                    if v <= 0:
                        break
                    m_lo = (Llo[tA] - cap_lo) >= (Lhi[tA] - cap_hi)
                    dm, do = (deg_lo, deg_hi) if m_lo else (deg_hi, deg_lo)
                    Lm, Lo = (Llo, Lhi) if m_lo else (Lhi, Llo)
                    cm, co = (cap_lo, cap_hi) if m_lo else (cap_hi, cap_lo)
                    done = False
                    order_a = sorted(nt_nodes[tA], key=lambda n: -dm[n])
                    for u in sorted(tset, key=lambda t: Lm[t]):
                        if u == tA:
                            continue
                        for a in order_a:
                            for b in sorted(nt_nodes[u], key=lambda n: dm[n]):
                                d_m = dm[a] - dm[b]
                                d_o = do[a] - do[b]
                                if d_m <= 0:
                                    break
                                if (Lm[u] + d_m <= cm and Lo[u] + d_o <= co
                                        and Lo[tA] - d_o <= co):
                                    nt_nodes[tA].remove(a)
                                    nt_nodes[u].remove(b)
                                    nt_nodes[tA].append(b)
                                    nt_nodes[u].append(a)
                                    Lm[tA] -= d_m; Lm[u] += d_m
                                    Lo[tA] -= d_o; Lo[u] += d_o
                                    done = True
                                    break
                            if done:
                                break
                        if done:
                            break
                    if not done:
                        return False, moved
                else:
                    return False, moved
                for t in tset:
                    if list(tiles_nodes[(c, t)]) != nt_nodes[t]:
                        moved = True
                    tiles_nodes[(c, t)] = np.array(nt_nodes[t], np.int64)
        return True, moved

    for cap_l, cap_h in (((K_lo - 1) * P, (K_hi - 1) * P),
                         ((K_lo - 1) * P, K_hi * P),
                         (K_lo * P, (K_hi - 1) * P)):
        if cap_l < P or cap_h < P:
            continue
        ok, _ = try_repair(cap_l, cap_h)
        if ok:
            K_lo = cap_l // P
            K_hi = cap_h // P
            break

    # rebuild local_of and edge lists from (possibly) updated tiles_nodes
    for c in range(NC):
        for tl in range(NT):
            for s, v in enumerate(tiles_nodes[(c, tl)]):
                local_of[v] = tl * P + s
    r_of = row_of(core_of, local_of // P, local_of % P)
    e_tile = local_of[dst] // P
    e_slot = local_of[dst] % P
    for c in range(NC):
        m_c = e_core == c
        for tl in range(NT):
            m = m_c & (e_tile == tl)
            ml = m & lo_src
            mh = m & ~lo_src
            rl, sl_ = r_of[src[ml]], e_slot[ml]
            rh, sh_ = r_of[src[mh]] - HALF, e_slot[mh]
            lists[(c, tl)] = (rl, sl_, rh, sh_)
            assert len(rl) <= K_lo * P and len(rh) <= K_hi * P

    T = K_lo + K_hi
    assert K_lo * P <= 1024 and K_hi * P <= 1024, (K_lo, K_hi)

    gidx = np.zeros((NC, P, NT * T * 8), np.int16)
    drel = np.full((NC, P, NT * T), -1.0, np.float32)
    for c in range(NC):
        for tl in range(NT):
            rl, sl_, rh, sh_ = lists[(c, tl)]
            for half, K, rows, slots in ((0, K_lo, rl, sl_),
                                         (1, K_hi, rh, sh_)):
                n = len(rows)
                flat = np.zeros(K * P, np.int64)
                flat[:n] = rows
                off = tl * T * 8 + (K_lo * 8 if half else 0)
                gidx[c, :, off:off + K * 8] = _pack_idx(flat)
                dr = np.full(K * P, -1.0, np.float32)
                dr[:n] = slots
                tc0 = tl * T + (K_lo if half else 0)
                drel[c, :, tc0:tc0 + K] = dr.reshape(K, P).T

    node_order = np.full((NC, NPC), -1, np.int64)
    for c in range(NC):
        nodes = np.where(core_of == c)[0]
        node_order[c, local_of[nodes]] = nodes

    return dict(NPC=NPC, NT=NT, TBL=TBL, HALF=HALF, K_lo=K_lo, K_hi=K_hi,
                T=T, ctiles=ctiles, ct0=[int(x) for x in ct0], crow0=crow0,
                gidx=gidx, drel=drel, node_order=node_order,
                core_of=core_of, local_of=local_of)


# --------------------------------------------------------------------------
# bass program
# --------------------------------------------------------------------------

def _build_program(dims):
    import concourse.bass as bass
    import concourse.mybir as mybir
    import concourse.tile as tile
    from concourse import library_config
    from concourse.bass import _add_dep_helper
    import bass_rust as _br

    fp32 = mybir.dt.float32
    bf = mybir.dt.bfloat16
    i16 = mybir.dt.int16
    AX = mybir.AxisListType
    OP = mybir.AluOpType
    AF = mybir.ActivationFunctionType

    DIN = dims["DIN"]; HC = dims["HC"]; H = dims["H"]; CH = dims["CH"]
    CO = dims["CO"]
    NPC = dims["NPC"]; NT = dims["NT"]; TBL = dims["TBL"]
    HALF = dims["HALF"]
    K_lo = dims["K_lo"]; K_hi = dims["K_hi"]; T = dims["T"]
    ctiles = dims["ctiles"]; ct0 = dims["ct0"]; crow0 = dims["crow0"]
    KD = DIN // P
    KH = HC // P
    COP = 128                       # layer-2 table row = 256B (bf16)

    nc = bass.Bass(num_devices=NC)

    xT = nc.dram_tensor("xT", [P, KD, NPC], bf, kind="ExternalInput")
    w1 = nc.dram_tensor("w1", [P, KD, 2 * HC], bf, kind="ExternalInput")
    w2 = nc.dram_tensor("w2", [P, KH, 2 * CO], bf, kind="ExternalInput")
    CCOLS = P + P + HC + CO + HC + CO + 2 * CO
    consts = nc.dram_tensor("consts", [P, CCOLS], bf, kind="ExternalInput")
    constf = nc.dram_tensor("constf", [P, 1], fp32, kind="ExternalInput")
    gidx_d = nc.dram_tensor("gidx", [P, NT * T * 8], i16, kind="ExternalInput")
    drel_d = nc.dram_tensor("drel", [P, NT * T], bf, kind="ExternalInput")
    h2_out = nc.dram_tensor("h2o", [NPC, CO], fp32, kind="ExternalOutput")
    ls_out = nc.dram_tensor("lso", [NPC, CO], fp32, kind="ExternalOutput")

    with tile.TileContext(nc) as tc:
        with (
            tc.tile_pool(name="dram", bufs=1, space="DRAM") as dram,
            tc.tile_pool(name="cst", bufs=1) as cst,
        ):
            lib = nc.gpsimd.load_library(library_config.mlp)
            reg_klo = nc.gpsimd.to_reg(K_lo * P)
            reg_khi = nc.gpsimd.to_reg(K_hi * P)

            ctile = cst.tile([P, CCOLS], bf)
            nc.sync.dma_start(out=ctile[:], in_=consts[:])
            cftile = cst.tile([P, 1], fp32)
            nc.sync.dma_start(out=cftile[:], in_=constf[:])
            o = 0
            iota = ctile[:, o:o + P]; o += P
            ident = ctile[:, o:o + P]; o += P
            attB = ctile[:, o:o + HC]; o += HC
            att2B = ctile[:, o:o + CO]; o += CO
            b1B = ctile[:, o:o + HC]; o += HC
            b2B = ctile[:, o:o + CO]; o += CO
            w2sB = ctile[:, o:o + 2 * CO]; o += 2 * CO
            alpha = cftile[:, 0:1]

            w1_sb = cst.tile([P, KD, 2 * HC], bf)
            nc.sync.dma_start(out=w1_sb[:], in_=w1[:])
            w2_sb = cst.tile([P, KH, 2 * CO], bf)
            nc.sync.dma_start(out=w2_sb[:], in_=w2[:])
            gidx_sb = cst.tile([P, NT * T * 8], i16)
            nc.sync.dma_start(out=gidx_sb[:], in_=gidx_d[:])
            drel_sb = cst.tile([P, NT * T], bf)
            nc.sync.dma_start(out=drel_sb[:], in_=drel_d[:])

            xr1_all = cst.tile([P, NT, HC], bf)
            xr2_all = cst.tile([P, NT, CO], bf)
            xl1_all = cst.tile([P, NT, HC], bf)
            xl2_all = cst.tile([P, NT, CO], bf)

            ag1c = [dram.tile([ctiles[i] * P, HC], bf, name=f"ag1c{i}")
                    for i in range(NCHUNK)]
            ag2c = [dram.tile([ctiles[i] * P, COP], bf, name=f"ag2c{i}")
                    for i in range(NCHUNK)]
            tbl1h = [dram.tile([NC * ctiles[i] * P, HC], bf,
                                addr_space="Shared", name=f"tbl1h{i}")
                     for i in range(NCHUNK)]
            tbl2h = [dram.tile([NC * ctiles[i] * P, COP], bf,
                                addr_space="Shared", name=f"tbl2h{i}")
                     for i in range(NCHUNK)]

            def ag_chunk(i, agc, tblh):
                nc.gpsimd.collective_compute(
                    "AllGather", mybir.AluOpType.bypass,
                    replica_groups=[list(range(NC))],
                    ins=[agc[i][:].opt()], outs=[tblh[i][:].opt()],
                )

            # ================= phase A: layer-1 projections =================
            with (tc.tile_pool(name="sbA", bufs=3) as sb,
                  tc.tile_pool(name="psA", bufs=2, space="PSUM") as ps):
                for i in range(NCHUNK):
                    for tl in range(ct0[i], ct0[i + 1]):
                        xt = sb.tile([P, KD, P], bf, tag="xt")
                        nc.sync.dma_start(
                            out=xt[:], in_=xT[:, :, tl * P:(tl + 1) * P])
                        pA = ps.tile([P, 2 * HC], fp32, tag="mm", space="PSUM")
                        for k in range(KD):
                            nc.tensor.matmul(out=pA[:], lhsT=xt[:, k, :],
                                             rhs=w1_sb[:, k, :],
                                             start=(k == 0), stop=(k == KD - 1))
                        xlr = sb.tile([P, 2 * HC], bf, tag="xlr")
                        nc.vector.tensor_copy(out=xlr[:], in_=pA[:])
                        nc.vector.tensor_copy(out=xr1_all[:, tl, :],
                                              in_=xlr[:, HC:2 * HC])
                        nc.vector.tensor_copy(out=xl1_all[:, tl, :],
                                              in_=xlr[:, 0:HC])
                        lt = tl - ct0[i]
                        nc.sync.dma_start(
                            out=ag1c[i][lt * P:(lt + 1) * P, :],
                            in_=xlr[:, 0:HC])
                    ag_chunk(i, ag1c, tbl1h)

            # ================= phase B: layer-1 edges =======================
            with (tc.tile_pool(name="sbB", bufs=2) as sb,
                  tc.tile_pool(name="psB", bufs=2, space="PSUM") as ps):
                for i in range(NCHUNK):
                    for tl in range(ct0[i], ct0[i + 1]):
                        nt = tl
                        g = sb.tile([P, T, HC], bf, tag="g", bufs=3)
                        off = nt * T * 8
                        g1 = nc.gpsimd.dma_gather(
                            g[:, 0:K_lo, :], tbl1h[0][:],
                            gidx_sb[:, off:off + K_lo * 8],
                            K_lo * P, reg_klo, HC)
                        g2 = nc.gpsimd.dma_gather(
                            g[:, K_lo:T, :], tbl1h[1][:],
                            gidx_sb[:, off + K_lo * 8:off + T * 8],
                            K_hi * P, reg_khi, HC)
                        _add_dep_helper(g1.ins, lib.ins, sync=False, reason="lib")
                        _add_dep_helper(g2.ins, lib.ins, sync=False, reason="lib")

                        oh = sb.tile([P, T, P], bf, tag="oh")
                        nc.vector.tensor_tensor(
                            out=oh[:],
                            in0=iota[:, None, :].to_broadcast([P, T, P]),
                            in1=drel_sb[:, nt * T:(nt + 1) * T, None]
                                .to_broadcast([P, T, P]),
                            op=OP.is_equal)

                        z = sb.tile([P, T, HC], bf, tag="z")
                        for g0 in range(0, T, 4):
                            gn = min(4, T - g0)
                            tp4 = ps.tile([P, 4, P], bf, tag="tp4",
                                          space="PSUM")
                            for j in range(gn):
                                nc.tensor.transpose(tp4[:, j, :],
                                                    oh[:, g0 + j, :], ident)
                            ohn4 = sb.tile([P, 4, P], bf, tag="ohn4")
                            nc.scalar.copy(out=ohn4[:, 0:gn, :],
                                           in_=tp4[:, 0:gn, :])
                            zr4 = ps.tile([P, 4, HC], fp32, tag="zr4",
                                          space="PSUM")
                            for j in range(gn):
                                nc.tensor.matmul(out=zr4[:, j, :],
                                                 lhsT=ohn4[:, j, :],
                                                 rhs=xr1_all[:, nt, :],
                                                 start=True, stop=False)
                                nc.tensor.matmul(out=zr4[:, j, :], lhsT=ident,
                                                 rhs=g[:, g0 + j, :],
                                                 start=False, stop=True)
                            nc.scalar.copy(out=z[:, g0:g0 + gn, :],
                                           in_=zr4[:, 0:gn, :])

                        pr = sb.tile([P, T, HC], bf, tag="pr")
                        nc.scalar.activation(out=pr[:], in_=z[:], func=AF.Prelu,
                                             alpha=alpha)
                        ta = sb.tile([P, T, HC], bf, tag="ta")
                        nc.vector.tensor_tensor(
                            out=ta[:], in0=pr[:],
                            in1=attB[:, None, :].to_broadcast([P, T, HC]),
                            op=OP.mult)
                        sc = sb.tile([P, T, H], fp32, tag="sc")
                        nc.vector.tensor_reduce(
                            out=sc[:],
                            in_=ta[:].rearrange("p t (h c) -> p t h c", h=H),
                            axis=AX.X, op=OP.add)
                        msg = sb.tile([P, T, HC + H], bf, tag="msg")
                        nc.scalar.activation(out=msg[:, :, HC:HC + H], in_=sc[:],
                                             func=AF.Exp)
                        nc.vector.tensor_tensor(
                            out=msg[:, :, 0:HC]
                                .rearrange("p t (h c) -> p t h c", h=H),
                            in0=g[:].rearrange("p t (h c) -> p t h c", h=H),
                            in1=msg[:, :, HC:HC + H][:, :, :, None]
                                .to_broadcast([P, T, H, CH]),
                            op=OP.mult)

                        # self-loop contribution: local, affine
                        zs = sb.tile([P, HC], bf, tag="zs")
                        nc.vector.tensor_tensor(out=zs[:],
                                                in0=xl1_all[:, nt, :],
                                                in1=xr1_all[:, nt, :],
                                                op=OP.add)
                        prs = sb.tile([P, HC], bf, tag="prs")
                        nc.scalar.activation(out=prs[:], in_=zs[:],
                                             func=AF.Prelu, alpha=alpha)
                        tas = sb.tile([P, HC], bf, tag="tas")
                        nc.vector.tensor_tensor(out=tas[:], in0=prs[:],
                                                in1=attB, op=OP.mult)
                        scs = sb.tile([P, H], fp32, tag="scs")
                        nc.vector.tensor_reduce(
                            out=scs[:],
                            in_=tas[:].rearrange("p (h c) -> p h c", h=H),
                            axis=AX.X, op=OP.add)
                        msgs = sb.tile([P, HC + H], bf, tag="msgs")
                        nc.scalar.activation(out=msgs[:, HC:HC + H],
                                             in_=scs[:], func=AF.Exp)
                        nc.vector.tensor_tensor(
                            out=msgs[:, 0:HC]
                                .rearrange("p (h c) -> p h c", h=H),
                            in0=xl1_all[:, nt, :]
                                .rearrange("p (h c) -> p h c", h=H),
                            in1=msgs[:, HC:HC + H][:, :, None]
                                .to_broadcast([P, H, CH]),
                            op=OP.mult)

                        acc = ps.tile([P, HC + H], fp32, tag="acc", space="PSUM", bufs=1)
                        nc.tensor.matmul(out=acc[:], lhsT=ident, rhs=msgs[:],
                                         start=True, stop=False)
                        for t in range(T):
                            nc.tensor.matmul(out=acc[:], lhsT=oh[:, t, :],
                                             rhs=msg[:, t, :],
                                             start=False, stop=(t == T - 1))

                        rec = sb.tile([P, H], fp32, tag="rec")
                        nc.vector.reciprocal(out=rec[:], in_=acc[:, HC:HC + H])
                        h1 = sb.tile([P, HC], bf, tag="h1")
                        nc.vector.tensor_tensor(
                            out=h1[:].rearrange("p (h c) -> p h c", h=H),
                            in0=acc[:, 0:HC].rearrange("p (h c) -> p h c", h=H),
                            in1=rec[:, :, None].to_broadcast([P, H, CH]),
                            op=OP.mult)
                        if dims["add_b1"]:
                            nc.vector.tensor_tensor(out=h1[:], in0=h1[:],
                                                    in1=b1B, op=OP.add)
                        # elu(h1)+1 = relu(h1) + min(exp(h1), 1)
                        el = sb.tile([P, HC], bf, tag="el")
                        nc.scalar.activation(out=el[:], in_=h1[:], func=AF.Relu)
                        ev = sb.tile([P, HC], bf, tag="ev")
                        nc.scalar.activation(out=ev[:], in_=h1[:], func=AF.Exp)
                        nc.vector.tensor_scalar(out=ev[:], in0=ev[:],
                                                scalar1=1.0, scalar2=None,
                                                op0=OP.min)
                        nc.vector.tensor_tensor(out=el[:], in0=el[:], in1=ev[:],
                                                op=OP.add)

                        hT = sb.tile([P, KH, P], bf, tag="hT")
                        tp4h = ps.tile([P, 4, P], bf, tag="tp4", space="PSUM")
                        for k in range(KH):
                            nc.tensor.transpose(tp4h[:, k, :],
                                                el[:, k * P:(k + 1) * P], ident)
                        nc.vector.tensor_copy(out=hT[:], in_=tp4h[:, 0:KH, :])
                        p2 = ps.tile([P, 2 * CO], fp32, tag="p2", space="PSUM", bufs=1)
                        for k in range(KH):
                            nc.tensor.matmul(out=p2[:], lhsT=hT[:, k, :],
                                             rhs=w2_sb[:, k, :],
                                             start=(k == 0), stop=(k == KH - 1))
                        xlr2 = sb.tile([P, 2 * CO], bf, tag="xlr2")
                        nc.vector.tensor_tensor(out=xlr2[:], in0=p2[:],
                                                in1=w2sB, op=OP.subtract)
                        nc.vector.tensor_copy(out=xr2_all[:, nt, :],
                                              in_=xlr2[:, CO:2 * CO])
                        nc.vector.tensor_copy(out=xl2_all[:, nt, :],
                                              in_=xlr2[:, 0:CO])
                        lt = tl - ct0[i]
                        nc.sync.dma_start(
                            out=ag2c[i][lt * P:(lt + 1) * P, 0:CO],
                            in_=xlr2[:, 0:CO])
                    ag_chunk(i, ag2c, tbl2h)

            # ================= phase C: layer-2 edges =======================
            with (tc.tile_pool(name="sbC", bufs=2) as sb,
                  tc.tile_pool(name="psC", bufs=2, space="PSUM") as ps):
                for nt in range(NT):
                    g2t = sb.tile([P, T, COP], bf, tag="g2", bufs=3)
                    off = nt * T * 8
                    g1 = nc.gpsimd.dma_gather(
                        g2t[:, 0:K_lo, :], tbl2h[0][:],
                        gidx_sb[:, off:off + K_lo * 8],
                        K_lo * P, reg_klo, COP)
                    g2 = nc.gpsimd.dma_gather(
                        g2t[:, K_lo:T, :], tbl2h[1][:],
                        gidx_sb[:, off + K_lo * 8:off + T * 8],
                        K_hi * P, reg_khi, COP)
                    _add_dep_helper(g1.ins, lib.ins, sync=False, reason="lib")
                    _add_dep_helper(g2.ins, lib.ins, sync=False, reason="lib")

                    oh = sb.tile([P, T, P], bf, tag="oh")
                    nc.vector.tensor_tensor(
                        out=oh[:],
                        in0=iota[:, None, :].to_broadcast([P, T, P]),
                        in1=drel_sb[:, nt * T:(nt + 1) * T, None]
                            .to_broadcast([P, T, P]),
                        op=OP.is_equal)

                    z2 = sb.tile([P, T, CO], bf, tag="z2")
                    for g0 in range(0, T, 4):
                        gn = min(4, T - g0)
                        tp4 = ps.tile([P, 4, P], bf, tag="tp4", space="PSUM")
                        for j in range(gn):
                            nc.tensor.transpose(tp4[:, j, :],
                                                oh[:, g0 + j, :], ident)
                        ohn4 = sb.tile([P, 4, P], bf, tag="ohn4")
                        nc.scalar.copy(out=ohn4[:, 0:gn, :],
                                       in_=tp4[:, 0:gn, :])
                        zr4 = ps.tile([P, 4, CO], fp32, tag="zr4",
                                      space="PSUM")
                        for j in range(gn):
                            nc.tensor.matmul(out=zr4[:, j, :],
                                             lhsT=ohn4[:, j, :],
                                             rhs=xr2_all[:, nt, :],
                                             start=True, stop=False)
                            nc.tensor.matmul(out=zr4[:, j, :], lhsT=ident,
                                             rhs=g2t[:, g0 + j, 0:CO],
                                             start=False, stop=True)
                        nc.scalar.copy(out=z2[:, g0:g0 + gn, :],
                                       in_=zr4[:, 0:gn, :])

                    pr2 = sb.tile([P, T, CO], bf, tag="pr2")
                    nc.scalar.activation(out=pr2[:], in_=z2[:], func=AF.Prelu,
                                         alpha=alpha)
                    ta2 = sb.tile([P, T, CO], bf, tag="ta2")
                    nc.vector.tensor_tensor(
                        out=ta2[:], in0=pr2[:],
                        in1=att2B[:, None, :].to_broadcast([P, T, CO]),
                        op=OP.mult)
                    sc2 = sb.tile([P, T], fp32, tag="sc2")
                    nc.vector.tensor_reduce(out=sc2[:], in_=ta2[:],
                                            axis=AX.X, op=OP.add)
                    msg2 = sb.tile([P, T, CO + 1], bf, tag="msg2")
                    nc.scalar.activation(out=msg2[:, :, CO:CO + 1],
                                         in_=sc2[:, :, None], func=AF.Exp)
                    nc.vector.tensor_tensor(
                        out=msg2[:, :, 0:CO], in0=g2t[:, :, 0:CO],
                        in1=msg2[:, :, CO:CO + 1].to_broadcast([P, T, CO]),
                        op=OP.mult)

                    zs2 = sb.tile([P, CO], bf, tag="zs2")
                    nc.vector.tensor_tensor(out=zs2[:], in0=xl2_all[:, nt, :],
                                            in1=xr2_all[:, nt, :], op=OP.add)
                    prs2 = sb.tile([P, CO], bf, tag="prs2")
                    nc.scalar.activation(out=prs2[:], in_=zs2[:],
                                         func=AF.Prelu, alpha=alpha)
                    tas2 = sb.tile([P, CO], bf, tag="tas2")
                    nc.vector.tensor_tensor(out=tas2[:], in0=prs2[:],
                                            in1=att2B, op=OP.mult)
                    scs2 = sb.tile([P, 1], fp32, tag="scs2")
                    nc.vector.tensor_reduce(out=scs2[:], in_=tas2[:],
                                            axis=AX.X, op=OP.add)
                    msgs2 = sb.tile([P, CO + 1], bf, tag="msgs2")
                    nc.scalar.activation(out=msgs2[:, CO:CO + 1],
                                         in_=scs2[:], func=AF.Exp)
                    nc.vector.tensor_tensor(
                        out=msgs2[:, 0:CO], in0=xl2_all[:, nt, :],
                        in1=msgs2[:, CO:CO + 1].to_broadcast([P, CO]),
                        op=OP.mult)

                    acc2 = ps.tile([P, CO + 1], fp32, tag="acc", space="PSUM", bufs=1)
                    nc.tensor.matmul(out=acc2[:], lhsT=ident, rhs=msgs2[:],
                                     start=True, stop=False)
                    for t in range(T):
                        nc.tensor.matmul(out=acc2[:], lhsT=oh[:, t, :],
                                         rhs=msg2[:, t, :],
                                         start=False, stop=(t == T - 1))

                    rec2 = sb.tile([P, 1], fp32, tag="rec2")
                    nc.vector.reciprocal(out=rec2[:], in_=acc2[:, CO:CO + 1])
                    h2 = sb.tile([P, CO], fp32, tag="h2")
                    nc.vector.tensor_scalar(out=h2[:], in0=acc2[:, 0:CO],
                                            scalar1=rec2[:, 0:1], scalar2=None,
                                            op0=OP.mult)
                    if dims["add_b2"]:
                        nc.vector.tensor_tensor(out=h2[:], in0=h2[:], in1=b2B,
                                                op=OP.add)
                    nc.sync.dma_start(out=h2_out[nt * P:(nt + 1) * P, :],
                                      in_=h2[:])
                    nm = sb.tile([P, 1], fp32, tag="nm")
                    nc.vector.tensor_reduce(out=nm[:], in_=h2[:], axis=AX.X,
                                            op=OP.max, negate=True)
                    esc = sb.tile([P, CO], fp32, tag="esc")
                    ssum = sb.tile([P, 1], fp32, tag="ssum")
                    nc.scalar.activation(out=esc[:], in_=h2[:], func=AF.Exp,
                                         bias=nm[:, 0:1],
                                         accum_out=ssum[:, 0:1])
                    lns = sb.tile([P, 1], fp32, tag="lns")
                    nc.scalar.activation(out=lns[:], in_=ssum[:], func=AF.Ln)
                    ls = sb.tile([P, CO], fp32, tag="ls")
                    nc.vector.tensor_tensor(out=ls[:], in0=h2[:],
                                            in1=nm[:, 0:1]
                                            .to_broadcast([P, CO]), op=OP.add)
                    nc.vector.tensor_tensor(out=ls[:], in0=ls[:],
                                            in1=lns[:, 0:1]
                                            .to_broadcast([P, CO]),
                                            op=OP.subtract)
                    nc.sync.dma_start(out=ls_out[nt * P:(nt + 1) * P, :],
                                      in_=ls[:])

    _br.generate_event_semaphores(nc)
    _br.codegen_inst_isa_subclasses(nc)
    return nc


# --------------------------------------------------------------------------
# entry point
# --------------------------------------------------------------------------

def kernel(x, edge_index, W1l, W1r, att1, b1, W2l, W2r, att2, b2):
    x = np.asarray(x, np.float32)
    edge_index = np.asarray(edge_index)
    W1l = np.asarray(W1l, np.float32); W1r = np.asarray(W1r, np.float32)
    att1 = np.asarray(att1, np.float32); b1 = np.asarray(b1, np.float32)
    W2l = np.asarray(W2l, np.float32); W2r = np.asarray(W2r, np.float32)
    att2 = np.asarray(att2, np.float32); b2 = np.asarray(b2, np.float32)

    N, DIN = x.shape
    E = edge_index.shape[1]
    H, CH = att1.shape
    HC = W1l.shape[1]
    CO = W2l.shape[1]
    KD = DIN // P
    KH = HC // P

    key = (N, E, DIN, H, CH, HC, CO,
           int(np.abs(b1).max() > 0), int(np.abs(b2).max() > 0),
           hash(edge_index.tobytes()))
    if key in _plan_cache:
        pp, nc, dims = _plan_cache[key]
    else:
        pp = _preprocess(N, E, edge_index)
        dims = dict(DIN=DIN, HC=HC, H=H, CH=CH, CO=CO,
                    NPC=pp["NPC"], NT=pp["NT"], TBL=pp["TBL"],
                    HALF=pp["HALF"], K_lo=pp["K_lo"], K_hi=pp["K_hi"],
                    T=pp["T"], ctiles=pp["ctiles"], ct0=pp["ct0"],
                    crow0=pp["crow0"],
                    add_b1=bool(np.abs(b1).max() > 0),
                    add_b2=bool(np.abs(b2).max() > 0))
        nc = _build_program(dims)
        _plan_cache[key] = (pp, nc, dims)

    NPC = pp["NPC"]
    bfdt = ml_dtypes.bfloat16

    iota = np.broadcast_to(np.arange(P, dtype=np.float32)[None, :], (P, P))
    ident = np.eye(P, dtype=np.float32)
    attB = np.broadcast_to(att1.reshape(1, HC), (P, HC))
    att2B = np.broadcast_to(att2.reshape(1, CO), (P, CO))
    b1B = np.broadcast_to(b1.reshape(1, HC), (P, HC))
    b2B = np.broadcast_to(b2.reshape(1, CO), (P, CO))
    w2s = np.concatenate([W2l.sum(axis=0), W2r.sum(axis=0)])
    w2sB = np.broadcast_to(w2s.reshape(1, 2 * CO), (P, 2 * CO))
    consts = np.concatenate([iota, ident, attB, att2B, b1B, b2B, w2sB],
                            axis=1).astype(bfdt)
    constf = np.full((P, 1), NEG_SLOPE, np.float32)

    w1p = np.concatenate([W1l.reshape(KD, P, HC).transpose(1, 0, 2),
                          W1r.reshape(KD, P, HC).transpose(1, 0, 2)],
                         axis=2).astype(bfdt)       # [P, KD, 2HC]
    w2p = np.concatenate([W2l.reshape(KH, P, CO).transpose(1, 0, 2),
                          W2r.reshape(KH, P, CO).transpose(1, 0, 2)],
                         axis=2).astype(bfdt)       # [P, KH, 2CO]

    in_maps = []
    for c in range(NC):
        sel = pp["node_order"][c]
        real = sel >= 0
        xa = np.zeros((NPC, DIN), np.float32)
        xa[real] = x[sel[real]]
        xTc = np.ascontiguousarray(
            xa.T.reshape(KD, P, NPC).transpose(1, 0, 2)).astype(bfdt)
        in_maps.append(dict(
            xT=xTc, w1=w1p, w2=w2p, consts=consts, constf=constf,
            gidx=np.ascontiguousarray(pp["gidx"][c]),
            drel=np.ascontiguousarray(pp["drel"][c]).astype(bfdt),
        ))

    from concourse.bass_utils import run_bass_kernel_spmd
    res = run_bass_kernel_spmd(nc, in_maps, core_ids=list(range(NC)))

    h = np.empty((N, CO), np.float32)
    ls = np.empty((N, CO), np.float32)
    r_core = pp["core_of"]
    r_loc = pp["local_of"]
    for c in range(NC):
        m = r_core == c
        h[m] = res.results[c]["h2o"][r_loc[m]]
        ls[m] = res.results[c]["lso"][r_loc[m]]
    return h, ls
